# revision 13
# baseline (speedup 1.0000x reference)
"""NeuroPredessor GNN message-passing kernel for 8 Trainium2 NeuronCores.

Sharding: core c owns nodes [1050c, 1050(c+1)) and pure vars
[8400+450c, 8400+450(c+1)).  Local var layout (free dim): 1050 node cols |
450 pure cols | 36 zero pad = 1536 cols.  All states are kept
feature-on-partition ([128, cols]) so every MLP/LSTM layer is a natural
PE matmul with no transposes.  The third MLP layer is computed "flipped"
(lhsT = activations, rhs = W3^T) so its output lands var-on-partition,
ready to be the stationary operand of the big unpack matmuls after a
per-round AllGather.  MLP output biases are folded into the big matmuls
as rank-1 updates using precomputed row/col sums of unpack.

The 0/1 incidence matrix `unpack` is streamed from HBM in fp8e4 in the two
layouts the two big matmuls need (columns = the core's nodes for
child->parent, columns = the core's vars for parent->children).
"""

import sys

if "/opt/trn_rl_repo" not in sys.path:
    sys.path.insert(0, "/opt/trn_rl_repo")

import numpy as np
import ml_dtypes

BF16NP = ml_dtypes.bfloat16
FP8NP = ml_dtypes.float8_e4m3

NCORES = 8
DIM = 128
NV = 12000
NN = 8400
NPC = NN // NCORES          # 1050 nodes per core
VPC = (NV - NN) // NCORES   # 450 pure vars per core
LV = 1536                   # local var cols (1050 + 450 + 36 pad), 12 tiles
LNP = 1152                  # local node cols padded to 9 tiles
KA = NCORES * LV            # 12288 contraction rows for c2p
KB = NCORES * LNP           # 9216 contraction rows for p2c
KTA = KA // 128             # 96
KTB = KB // 128             # 72
ROUNDS = 12

TRACE = False
LAST_EXEC_NS = None
_CACHE = {}


def _chunks(width, step=512):
    return [(j, min(j + step, width)) for j in range(0, width, step)]


def _build(rounds=ROUNDS):
    import concourse.bass as bass
    import concourse.bacc as bacc
    import concourse.tile as tile
    import concourse.mybir as mybir
    from contextlib import ExitStack

    F32 = mybir.dt.float32
    BF = mybir.dt.bfloat16
    F8 = mybir.dt.float8e4
    F16 = mybir.dt.float16
    AF = mybir.ActivationFunctionType

    nc = bacc.Bacc("TRN2", target_bir_lowering=False, debug=False,
                   num_devices=NCORES)

    d_upA = nc.dram_tensor("upA", [KA, NPC], F8, kind="ExternalInput")
    d_upB = nc.dram_tensor("upB", [KB, LV], F8, kind="ExternalInput")
    d_h0 = nc.dram_tensor("h0T", [DIM, LV], F32, kind="ExternalInput")
    d_wbf = nc.dram_tensor("wbf", [DIM, 3073], F16, kind="ExternalInput")
    d_bf32 = nc.dram_tensor("bf32", [DIM, 15], F32, kind="ExternalInput")
    d_rows = nc.dram_tensor("rows", [1, 2842], F16, kind="ExternalInput")
    d_out = nc.dram_tensor("vote_out", [1, VPC], F32, kind="ExternalOutput")

    RG = [list(range(NCORES))]

    with tile.TileContext(nc) as tc, ExitStack() as ctx:
        const = ctx.enter_context(tc.tile_pool(name="const", bufs=1))
        work = ctx.enter_context(tc.tile_pool(name="work", bufs=2))
        stream = ctx.enter_context(tc.tile_pool(name="stream", bufs=10))
        gathp = ctx.enter_context(tc.tile_pool(name="gath", bufs=1))
        pbig = ctx.enter_context(tc.tile_pool(name="pbig", bufs=1, space="PSUM"))
        pg = ctx.enter_context(tc.tile_pool(name="pg", bufs=1, space="PSUM"))
        dramp = ctx.enter_context(tc.tile_pool(name="dram", bufs=1, space="DRAM"))

        wbf = const.tile([DIM, 3073], F16)
        nc.sync.dma_start(wbf[:], d_wbf[:])
        bf32 = const.tile([DIM, 15], F32)
        nc.sync.dma_start(bf32[:], d_bf32[:])
        rows = const.tile([1, 2842], F16)
        nc.sync.dma_start(rows[:], d_rows[:])
        h = const.tile([DIM, LV], F32)
        nc.sync.dma_start(h[:], d_h0[:])
        cst = const.tile([DIM, LV], F32)
        nc.gpsimd.memset(cst[:], 0.0)
        hbf = const.tile([DIM, LV], F16)
        nc.scalar.activation(hbf[:], h[:], AF.Copy)

        # weight slices (wbf cols)
        cm_w1T, cm_w2T, cm_w3T = wbf[:, 0:128], wbf[:, 128:256], wbf[:, 256:384]
        pm_w1T, pm_w2T, pm_w3T = wbf[:, 384:512], wbf[:, 512:640], wbf[:, 640:768]
        vv_w1T, vv_w2T = wbf[:, 768:896], wbf[:, 896:1024]
        vu_wihT, vu_whhT = wbf[:, 1024:1536], wbf[:, 1536:2048]
        nu_wihT, nu_whhT = wbf[:, 2048:2560], wbf[:, 2560:3072]
        vv_w3T = wbf[:, 3072:3073]
        cm_b1, cm_b2 = bf32[:, 0:1], bf32[:, 1:2]
        pm_b1, pm_b2 = bf32[:, 2:3], bf32[:, 3:4]
        vv_b1, vv_b2 = bf32[:, 4:5], bf32[:, 5:6]
        vu_bias, nu_bias = bf32[:, 6:10], bf32[:, 10:14]
        vv_b3ap = bf32[:1, 14:15]
        cm_b3row, pm_b3row = rows[:, 0:128], rows[:, 128:256]
        rowsum = rows[:, 256:1306]          # [1, 1050]
        colsum = rows[:, 1306:2842]         # [1, 1536]

        def mlp2(xin, w1T, b1, w2T, b2, width):
            ps = pbig.tile([DIM, LV], F32, tag="pbig")
            for j0, j1 in _chunks(width):
                nc.tensor.matmul(ps[:, j0:j1], w1T, xin[:, j0:j1],
                                 start=True, stop=True)
            x1 = work.tile([DIM, width], F16, tag="x1")
            nc.scalar.activation(x1[:], ps[:, :width], AF.Relu, bias=b1)
            ps2 = pbig.tile([DIM, LV], F32, tag="pbig")
            for j0, j1 in _chunks(width):
                nc.tensor.matmul(ps2[:, j0:j1], w2T, x1[:, j0:j1],
                                 start=True, stop=True)
            x2 = work.tile([DIM, width], F16, tag="x2")
            nc.scalar.activation(x2[:], ps2[:, :width], AF.Relu, bias=b2)
            return x2

        def flip(x2, w3T, ntiles):
            out = work.tile([DIM, ntiles * 128], F16, tag="flip")
            for t in range(ntiles):
                fp = pg.tile([DIM, 128], F32, tag=f"g{t % 4}",
                             name=f"flip{t}")
                nc.tensor.matmul(fp[:], x2[:, 128 * t:128 * (t + 1)], w3T,
                                 start=True, stop=True)
                nc.scalar.activation(out[:, 128 * t:128 * (t + 1)], fp[:],
                                     AF.Copy)
            return out

        def allgather(local, wloc, name):
            din = dramp.tile([wloc, DIM], F16, tag=name + "_in")
            dout = dramp.tile([NCORES * wloc, DIM], F16, tag=name + "_out",
                              addr_space="Shared")
            nt = wloc // 128
            nc.sync.dma_start(
                din[:].rearrange("(t p) f -> p t f", p=128),
                local[:].rearrange("p (t f) -> p t f", t=nt))
            nc.gpsimd.collective_compute(
                "AllGather", mybir.AluOpType.bypass,
                ins=[din[:].opt()], outs=[dout[:].opt()], replica_groups=RG)
            g = gathp.tile([DIM, NCORES * wloc], F16, tag=name + "_g")
            nc.sync.dma_start(
                g[:].rearrange("p (t f) -> p t f", t=NCORES * nt),
                dout[:].rearrange("(t p) f -> p t f", p=128))
            return g

        def big_matmul(g, dsrc, ktiles, ncols, b3row, sums):
            ps = pbig.tile([DIM, LV], F32, tag="pbig")
            cks = _chunks(ncols)
            for kb in range(0, ktiles, 4):
                at = stream.tile([DIM, 4 * LV], F8, tag="up")
                nc.sync.dma_start(
                    at[:, :4 * ncols].rearrange("p (a n) -> p a n", a=4),
                    dsrc[128 * kb:128 * (kb + 4), :ncols].rearrange(
                        "(a p) n -> p a n", p=128))
                for a in range(4):
                    k = kb + a
                    lhsT = g[:, 128 * k:128 * (k + 1)]
                    for j0, j1 in cks:
                        nc.tensor.matmul(
                            ps[:, j0:j1], lhsT,
                            at[:, a * ncols + j0:a * ncols + j1],
                            start=(k == 0), stop=False)
            for j0, j1 in cks:
                nc.tensor.matmul(ps[:, j0:j1], b3row, sums[:, j0:j1],
                                 start=False, stop=True)
            return ps

        def lstm(xps, width, wihT, whhT, bias4, off):
            xbf = work.tile([DIM, width], F16, tag="xbf")
            nc.scalar.activation(xbf[:], xps[:, :width], AF.Copy)
            for j0, j1 in _chunks(width):
                w = j1 - j0
                gps = [pg.tile([DIM, 512], F32, tag=f"g{i}", name=f"g{i}")
                       for i in range(4)]
                for gi in range(4):
                    nc.tensor.matmul(gps[gi][:, :w],
                                     wihT[:, 128 * gi:128 * (gi + 1)],
                                     xbf[:, j0:j1], start=True, stop=False)
                    nc.tensor.matmul(gps[gi][:, :w],
                                     whhT[:, 128 * gi:128 * (gi + 1)],
                                     hbf[:, off + j0:off + j1],
                                     start=False, stop=True)
                i_s = work.tile([DIM, 512], F32, tag="i_s")
                nc.scalar.activation(i_s[:, :w], gps[0][:, :w], AF.Sigmoid,
                                     bias=bias4[:, 0:1])
                f_s = work.tile([DIM, 512], F32, tag="f_s")
                nc.scalar.activation(f_s[:, :w], gps[1][:, :w], AF.Sigmoid,
                                     bias=bias4[:, 1:2])
                g_t = work.tile([DIM, 512], F32, tag="g_t")
                nc.scalar.activation(g_t[:, :w], gps[2][:, :w], AF.Tanh,
                                     bias=bias4[:, 2:3])
                o_s = work.tile([DIM, 512], F32, tag="o_s")
                nc.scalar.activation(o_s[:, :w], gps[3][:, :w], AF.Sigmoid,
                                     bias=bias4[:, 3:4])
                t1 = work.tile([DIM, 512], F32, tag="t1")
                nc.vector.tensor_mul(t1[:, :w], f_s[:, :w],
                                     cst[:, off + j0:off + j1])
                t2 = work.tile([DIM, 512], F32, tag="t2")
                nc.vector.tensor_mul(t2[:, :w], i_s[:, :w], g_t[:, :w])
                nc.vector.tensor_add(cst[:, off + j0:off + j1],
                                     t1[:, :w], t2[:, :w])
                tc2 = work.tile([DIM, 512], F32, tag="tc2")
                nc.scalar.activation(tc2[:, :w], cst[:, off + j0:off + j1],
                                     AF.Tanh)
                nc.vector.tensor_mul(h[:, off + j0:off + j1],
                                     o_s[:, :w], tc2[:, :w])
                nc.scalar.activation(hbf[:, off + j0:off + j1],
                                     h[:, off + j0:off + j1], AF.Copy)

        for r in range(rounds):
            x2 = mlp2(hbf, cm_w1T, cm_b1, cm_w2T, cm_b2, LV)
            cml = flip(x2, cm_w3T, 12)
            cm_g = allgather(cml, LV, "cm")
            msgps = big_matmul(cm_g, d_upA, KTA, NPC, cm_b3row, rowsum)
            lstm(msgps, NPC, vu_wihT, vu_whhT, vu_bias, 0)
            x2p = mlp2(hbf, pm_w1T, pm_b1, pm_w2T, pm_b2, LNP)
            pml = flip(x2p, pm_w3T, 9)
            pm_g = allgather(pml, LNP, "pm")
            p2cps = big_matmul(pm_g, d_upB, KTB, 1500, pm_b3row, colsum)
            lstm(p2cps, 1500, nu_wihT, nu_whhT, nu_bias, 0)

        # vote MLP on pure vars (local cols 1050:1500)
        vps = pg.tile([DIM, 512], F32, tag="g0")
        nc.tensor.matmul(vps[:, :VPC], vv_w1T, hbf[:, NPC:NPC + VPC],
                         start=True, stop=True)
        v1 = work.tile([DIM, 512], F16, tag="x1")
        nc.scalar.activation(v1[:, :VPC], vps[:, :VPC], AF.Relu, bias=vv_b1)
        vps2 = pg.tile([DIM, 512], F32, tag="g1")
        nc.tensor.matmul(vps2[:, :VPC], vv_w2T, v1[:, :VPC],
                         start=True, stop=True)
        v2 = work.tile([DIM, 512], F16, tag="x2")
        nc.scalar.activation(v2[:, :VPC], vps2[:, :VPC], AF.Relu, bias=vv_b2)
        vps3 = pg.tile([DIM, 512], F32, tag="g2")
        nc.tensor.matmul(vps3[:1, :VPC], vv_w3T, v2[:, :VPC],
                         start=True, stop=True)
        vout = work.tile([1, 512], F32, tag="vout")
        nc.scalar.activation(vout[:1, :VPC], vps3[:1, :VPC], AF.Identity,
                             bias=vv_b3ap)
        nc.sync.dma_start(d_out[:], vout[:1, :VPC])

    nc.compile()
    return nc


def _prep_inputs(inp):
    U = np.asarray(inp["unpack"], np.float32)
    Ubf = U.astype(FP8NP)
    vt = np.asarray(inp["vt"])
    tval = (np.asarray(inp["true_w"]) + np.asarray(inp["true_b"])).astype(np.float32)
    fval = (np.asarray(inp["false_w"]) + np.asarray(inp["false_b"])).astype(np.float32)

    def T(x):
        return np.ascontiguousarray(np.asarray(x, np.float32).T)

    wbf_cols = [T(inp["cm_w1"]), T(inp["cm_w2"]), T(inp["cm_w3"]),
                T(inp["pm_w1"]), T(inp["pm_w2"]), T(inp["pm_w3"]),
                T(inp["vv_w1"]), T(inp["vv_w2"]),
                T(inp["vu_wih"]), T(inp["vu_whh"]),
                T(inp["nu_wih"]), T(inp["nu_whh"]), T(inp["vv_w3"])]
    wbf = np.concatenate(wbf_cols, axis=1).astype(np.float16)   # [128, 3073]

    def g4(bih, bhh):
        b = (np.asarray(bih, np.float32) + np.asarray(bhh, np.float32))
        return b.reshape(4, DIM).T                             # [128, 4]

    bf32 = np.concatenate(
        [np.asarray(inp[k], np.float32).reshape(DIM, 1)
         for k in ("cm_b1", "cm_b2", "pm_b1", "pm_b2", "vv_b1", "vv_b2")]
        + [g4(inp["vu_bih"], inp["vu_bhh"]), g4(inp["nu_bih"], inp["nu_bhh"]),
           np.full((DIM, 1), float(np.asarray(inp["vv_b3"]).reshape(-1)[0]),
                   np.float32)],
        axis=1).astype(np.float32)                             # [128, 15]

    cm_b3 = np.asarray(inp["cm_b3"], np.float32)
    pm_b3 = np.asarray(inp["pm_b3"], np.float32)

    in_maps = []
    for c in range(NCORES):
        nsl = slice(1050 * c, 1050 * (c + 1))
        psl = slice(NN + 450 * c, NN + 450 * (c + 1))
        A = np.zeros((KA, NPC), FP8NP)
        B = np.zeros((KB, LV), FP8NP)
        for b in range(NCORES):
            nb = slice(1050 * b, 1050 * (b + 1))
            pb = slice(NN + 450 * b, NN + 450 * (b + 1))
            A[1536 * b:1536 * b + 1050, :] = Ubf[nsl, nb].T
            A[1536 * b + 1050:1536 * b + 1500, :] = Ubf[nsl, pb].T
            B[1152 * b:1152 * b + 1050, 0:1050] = Ubf[nb, nsl]
            B[1152 * b:1152 * b + 1050, 1050:1500] = Ubf[nb, psl]
        rowsum = U[nsl, :].sum(1)                              # [1050]
        colsum = np.zeros(LV, np.float32)
        colsum[:1050] = U[:, nsl].sum(0)
        colsum[1050:1500] = U[:, psl].sum(0)
        rows = np.concatenate([cm_b3, pm_b3, rowsum, colsum]
                              ).reshape(1, 2842).astype(np.float16)
        vtl = np.concatenate([vt[nsl], vt[psl]])
        h0 = np.empty((DIM, LV), np.float32)
        h0[:, :1500] = np.where(vtl[None, :] == 1, tval[:, None], fval[:, None])
        h0[:, 1500:] = fval[:, None]
        in_maps.append({"upA": A, "upB": B, "h0T": h0, "wbf": wbf,
                        "bf32": bf32, "rows": rows})
    return in_maps


def kernel(**inputs):
    global LAST_EXEC_NS
    from concourse import bass_utils

    key = ("nc", ROUNDS)
    if key not in _CACHE:
        _CACHE[key] = _build(ROUNDS)
    nc = _CACHE[key]

    in_maps = _prep_inputs(inputs)
    res = bass_utils.run_bass_kernel_spmd(
        nc, in_maps, core_ids=list(range(NCORES)), trace=TRACE)
    LAST_EXEC_NS = res.exec_time_ns

    out = np.empty(NV - NN, np.float32)
    for c in range(NCORES):
        out[450 * c:450 * (c + 1)] = \
            np.asarray(res.results[c]["vote_out"], np.float32).reshape(-1)
    return out


if __name__ == "__main__":
    import reference
    inp = reference.setup_inputs()
    out = kernel(**{k: np.asarray(v) if not np.isscalar(v) else v
                    for k, v in inp.items()})
    print(out[:8], LAST_EXEC_NS)
